# revision 5
# baseline (speedup 1.0000x reference)
"""Trainium2 Bass kernel for nn_BAKTTime: causal-conv frequency layer + LN + causal MHA.

Sharding: pure data-parallel over batch - 8 of the 64 batch items per NeuronCore,
no collectives. Each core runs a 5-stage software-pipelined program over its 8
batch items (S=512, D=512, H=8, DK=64).

v2: the conv and Q/K projections run as fp8-e4m3 DoubleRow matmuls (2 fp8
K-blocks per instruction at 0.5 cycles/column = 4x bf16 throughput per pass),
with precision recovered by hi+lo operand splitting:
  - conv: 3-term split  W_hi@x_hi + W_lo@x_hi + W_hi@x_lo. Host pre-scales the
    folded conv weights by 8 so the W residual clears e4m3's subnormal floor;
    the LN that follows is scale-invariant, so no compensation is needed.
    The 12 (chunk, tap) x-window blocks are host-duplicated per s-tile into a
    dense [128, st, 6, 2, 128] layout because DoubleRow Ldweights requires a
    contiguous stationary AP.
  - q/k: 2-term split (W_hi + W_lo) @ h_hi. Weights are host-split (scaled by
    64, compensated inside the exp: scale 0.125/4096); h_hi is a single e4m3
    quantization of the transposed h (softmax flatness damps q/k noise).
  - v / scores / ctx / out-proj stay bf16 (their errors are undamped and the
    cost model's fp8 gain there is small).
Per-batch dataflow:
  1. conv per s-tile: 18 accumulating DoubleRow matmuls -> PSUM [s,d].
  2. LN per s-tile: bn_stats/bn_aggr straight from PSUM; rstd = exp(-.5 ln(var));
     h = (a-mean)*rstd fused into the PSUM->SBUF tensor_scalar (ln_w folded
     into QKV weights on host); ONE 128x512 transpose DMA per s-tile into
     ht[128, st, j, 128] (d = j*128+p); gpsimd splits ht -> ht_hi (+ht_lo unused
     by default).
  3. q/k: DoubleRow fp8 projections (4 instr/chunk); v: bf16 with a ones column
     per head (v_aug) so the ctx matmul also yields the softmax denominator.
  4. attention per head-PAIR: scoresT[k,q] for both heads in one [128, 2x512]
     PSUM tile (causal-trimmed); ONE exp (ACT, scale=0.125/4096 compensating
     the 64x q/k weight scaling, ->bf16) and ONE tri-mask multiply; ctx
     accumulates v_aug^T @ PT per head into [65, S] (row 64 = denominator).
  5. ctx evacuation: ACT copy into one cxu[65, 8, S] tile with a per-partition
     bias that adds 1e-10 only to the denominator row (zero_pad guard).
  6. denominator: one gpsimd SWDGE gather of row 64 -> [8,512] bf16,
     reciprocal_approx_fast (DVE), 4 pair-packed broadcast DMAs [2->128, S].
  7. normalize+pack on gpsimd into head-pair tiles [128, S] (odd head moved to
     partitions 64-127 by a partition-shifting DMA); out-proj bf16 K=128:
     16 matmuls -> PSUM -> ACT copy -> DRAM.
"""

import sys

sys.path.insert(0, "/opt/trn_rl_repo")

import numpy as np
import ml_dtypes
from contextlib import ExitStack

import concourse.bass as bass
from concourse import bacc
import concourse.mybir as mybir
import concourse.tile as tile
from concourse.bass_utils import run_bass_kernel_spmd

# Force Exp and Ln into the single table set containing both, so ACT never
# thrashes table loads (~2.7us per switch).
import concourse.hw_specs as _hw_specs

_orig_get_tables = _hw_specs.get_activation_tables


def _patched_get_tables(arch):
    t = dict(_orig_get_tables(arch))
    exp = mybir.ActivationFunctionType.Exp
    ln = mybir.ActivationFunctionType.Ln
    for name, funcs in t.items():
        if name != "natural_log_exp_and_others" and (exp in funcs or ln in funcs):
            t[name] = funcs - {exp, ln}
    return t


_hw_specs.get_activation_tables = _patched_get_tables
bacc.get_activation_tables = _patched_get_tables

B, S, D, H, KW = 64, 512, 512, 8, 3
DK = D // H  # 64
NCORES = 8
BL = B // NCORES  # 8 batches per core
P = 128
NST = S // P  # 4 s-tiles
NIC = D // P  # 4 input-chunks
NPAIR = (NIC * KW) // 2  # 6 DoubleRow block-pairs for the conv
EPS = 1e-12
WC_SCALE = 8.0  # conv weight pre-scale (LN-invariant)
WQK_SCALE = 64.0  # q/k weight pre-scale (compensated in exp scale)
EXP_SCALE = 0.125 / (WQK_SCALE * WQK_SCALE)
QK_THIRD_TERM = False  # add W_hi @ h_lo pass for q/k
F32 = mybir.dt.float32
BF16 = mybir.dt.bfloat16
FP8 = mybir.dt.float8e4
AF = mybir.ActivationFunctionType
DR = mybir.MatmulPerfMode.DoubleRow

# conv block raster: block b = (chunk i, tap k); DoubleRow pair m = blocks (2m, 2m+1)
BLOCKS = [(i, k) for i in range(NIC) for k in range(KW)]


def build_nc():
    nc = bacc.Bacc("TRN2", target_bir_lowering=False)
    xdr_hi = nc.declare_dram_parameter("xdr_hi", [BL, P, NST, NPAIR, 2, P], FP8, isOutput=False)
    xdr_lo = nc.declare_dram_parameter("xdr_lo", [BL, P, NST, NPAIR, 2, P], FP8, isOutput=False)
    wc_hi = nc.declare_dram_parameter("wc_hi", [P, NPAIR, 2, D], FP8, isOutput=False)
    wc_lo = nc.declare_dram_parameter("wc_lo", [P, NPAIR, 2, D], FP8, isOutput=False)
    wq_hi = nc.declare_dram_parameter("wq_hi", [P, 2, NIC, 2, P], FP8, isOutput=False)
    wq_lo = nc.declare_dram_parameter("wq_lo", [P, 2, NIC, 2, P], FP8, isOutput=False)
    wk_hi = nc.declare_dram_parameter("wk_hi", [P, 2, NIC, 2, P], FP8, isOutput=False)
    wk_lo = nc.declare_dram_parameter("wk_lo", [P, 2, NIC, 2, P], FP8, isOutput=False)
    wv = nc.declare_dram_parameter("wv", [NIC, P, D], BF16, isOutput=False)
    wo = nc.declare_dram_parameter("wo", [NIC, P, D], BF16, isOutput=False)
    trim = nc.declare_dram_parameter("trim", [P, 2, P], BF16, isOutput=False)
    out = nc.declare_dram_parameter("out", [BL, S, D], F32, isOutput=True)

    with ExitStack() as ctx:
        tc = ctx.enter_context(tile.TileContext(nc))
        singles = ctx.enter_context(tc.tile_pool(name="singles", bufs=1))
        x_pool = ctx.enter_context(tc.tile_pool(name="x", bufs=2))
        stat_pool = ctx.enter_context(tc.tile_pool(name="stat", bufs=8))
        h_pool = ctx.enter_context(tc.tile_pool(name="h", bufs=4))
        ht_pool = ctx.enter_context(tc.tile_pool(name="ht", bufs=2))
        qk_pool = ctx.enter_context(tc.tile_pool(name="qk", bufs=16))
        v_pool = ctx.enter_context(tc.tile_pool(name="v", bufs=8))
        pt_pool = ctx.enter_context(tc.tile_pool(name="pt", bufs=6))
        cxa_pool = ctx.enter_context(tc.tile_pool(name="cxa", bufs=2))
        dn_pool = ctx.enter_context(tc.tile_pool(name="dn", bufs=2))
        r_pool = ctx.enter_context(tc.tile_pool(name="r", bufs=8))
        cx_pool = ctx.enter_context(tc.tile_pool(name="cx", bufs=16))
        o_pool = ctx.enter_context(tc.tile_pool(name="o", bufs=6))
        ps_a = ctx.enter_context(tc.tile_pool(name="ps_a", bufs=2, space="PSUM"))
        ps_mm = ctx.enter_context(tc.tile_pool(name="ps_mm", bufs=2, space="PSUM"))
        ps_sc = ctx.enter_context(tc.tile_pool(name="ps_sc", bufs=1, space="PSUM"))
        ps_cx = ctx.enter_context(tc.tile_pool(name="ps_cx", bufs=2, space="PSUM"))

        # --- load weights once ---
        wch_sb = singles.tile([P, NPAIR, 2, D], FP8, name="wch", tag="wch")
        wcl_sb = singles.tile([P, NPAIR, 2, D], FP8, name="wcl", tag="wcl")
        wqh_sb = singles.tile([P, 2, NIC, 2, P], FP8, name="wqh", tag="wqh")
        wql_sb = singles.tile([P, 2, NIC, 2, P], FP8, name="wql", tag="wql")
        wkh_sb = singles.tile([P, 2, NIC, 2, P], FP8, name="wkh", tag="wkh")
        wkl_sb = singles.tile([P, 2, NIC, 2, P], FP8, name="wkl", tag="wkl")
        wv_sb = [singles.tile([P, D], BF16, name=f"wv{i}", tag=f"wv{i}") for i in range(NIC)]
        wo_sb = [singles.tile([P, D], BF16, name=f"wo{i}", tag=f"wo{i}") for i in range(NIC)]
        trim_sb = singles.tile([P, 2, P], BF16, name="trim", tag="trim")
        eps_sb = singles.tile([P, 1], F32, name="eps", tag="eps")
        nc.vector.memset(eps_sb, EPS)
        zero_sb = singles.tile([P, 1], F32, name="zero", tag="zero")
        nc.vector.memset(zero_sb, 0.0)
        tiny65 = singles.tile([65, 1], F32, name="tiny65", tag="tiny65")
        nc.vector.memset(tiny65, 0.0)
        nc.vector.memset(tiny65[64:65, :], 1e-10)
        # conv weights + trim first (needed by iteration 0)
        nc.gpsimd.dma_start(out=wch_sb, in_=wc_hi[:])
        nc.gpsimd.dma_start(out=wcl_sb, in_=wc_lo[:])
        nc.gpsimd.dma_start(out=trim_sb, in_=trim[:])
        nc.gpsimd.dma_start(out=wqh_sb, in_=wq_hi[:])
        nc.gpsimd.dma_start(out=wql_sb, in_=wq_lo[:])
        nc.gpsimd.dma_start(out=wkh_sb, in_=wk_hi[:])
        nc.gpsimd.dma_start(out=wkl_sb, in_=wk_lo[:])
        for i in range(NIC):
            nc.gpsimd.dma_start(out=wv_sb[i], in_=wv[i])
        for i in range(NIC):
            nc.gpsimd.dma_start(out=wo_sb[i], in_=wo[i])

        def load_x(b):
            xh = x_pool.tile([P, NST, NPAIR, 2, P], FP8, name="xh", tag="xh")
            xl = x_pool.tile([P, NST, NPAIR, 2, P], FP8, name="xl", tag="xl")
            nc.sync.dma_start(out=xh, in_=xdr_hi[b])
            nc.sync.dma_start(out=xl, in_=xdr_lo[b])
            return (xh, xl)

        def front(b, xs):
            """conv + LN + transpose + fp8 split for batch b."""
            xh, xl = xs
            ht_all = ht_pool.tile([P, NST, NIC, P], BF16, name="hta", tag="hta")
            for st in range(NST):
                aps = ps_a.tile([P, D], F32, name="aps", tag="aps")
                passes = [(wch_sb, xh), (wcl_sb, xh), (wch_sb, xl)]
                n = len(passes) * NPAIR
                idx = 0
                for wsb, xsb in passes:
                    for m in range(NPAIR):
                        nc.tensor.matmul(
                            aps,
                            lhsT=xsb[:, st, m, :, :],
                            rhs=wsb[:, m, :, :],
                            start=(idx == 0),
                            stop=(idx == n - 1),
                            perf_mode=DR,
                        )
                        idx += 1
                stats = stat_pool.tile([P, 6], F32, name="bnst", tag="bnst")
                nc.vector.bn_stats(out=stats, in_=aps)
                mv = stat_pool.tile([P, 2], F32, name="mv", tag="mv")
                nc.vector.bn_aggr(out=mv, in_=stats)
                lnv = stat_pool.tile([P, 1], F32, name="lnv", tag="lnv")
                nc.scalar.activation(lnv, mv[:, 1:2], AF.Ln, bias=eps_sb, scale=1.0)
                rstd = stat_pool.tile([P, 1], F32, name="rstd", tag="rstd")
                nc.scalar.activation(rstd, lnv, AF.Exp, bias=zero_sb, scale=-0.5)
                hsb = h_pool.tile([P, D], BF16, name="hsb", tag="hsb")
                nc.vector.tensor_scalar(
                    hsb,
                    aps,
                    scalar1=mv[:, 0:1],
                    scalar2=rstd,
                    op0=mybir.AluOpType.subtract,
                    op1=mybir.AluOpType.mult,
                )
                nc.sync.dma_start(out=ht_all[:, st, :, :], in_=hsb, transpose=True)
            ht_hi = ht_pool.tile([P, NST, NIC, P], FP8, name="hth", tag="hth")
            nc.gpsimd.tensor_copy(ht_hi, ht_all)
            if QK_THIRD_TERM:
                ht_lo = ht_pool.tile([P, NST, NIC, P], FP8, name="htl", tag="htl")
                nc.gpsimd.tensor_tensor(
                    out=ht_lo, in0=ht_all, in1=ht_hi, op=mybir.AluOpType.subtract
                )
            else:
                ht_lo = None
            return (ht_all, ht_hi, ht_lo)

        def qk_rhs(hsb8, kblk):
            # moving AP over ht[p, st, j, c]: (j in pair, st, c) -> [K, 2, S]
            base = hsb8[0:P, 0, 0, 0]
            return bass.AP(
                tensor=base.tensor,
                offset=base.offset + 2 * kblk * P,
                ap=[base.ap[0], [P, 2], [NIC * P, NST], [1, P]],
            )

        def mid(b, hts):
            """projections + attention for batch b. Returns tail state."""
            ht_all, ht_hi, ht_lo = hts
            qt_sb = []
            kt_sb = []
            for wh, wl, dst in ((wqh_sb, wql_sb, qt_sb), (wkh_sb, wkl_sb, kt_sb)):
                for oc in range(NIC):
                    qps = ps_mm.tile([P, S], F32, name="qps", tag="qps")
                    passes = [(wh, ht_hi), (wl, ht_hi)]
                    if QK_THIRD_TERM:
                        passes.append((wh, ht_lo))
                    n = 2 * len(passes)
                    idx = 0
                    for wsb, hsb8 in passes:
                        for kblk in range(2):
                            nc.tensor.matmul(
                                qps,
                                lhsT=wsb[:, kblk, oc, :, :],
                                rhs=qk_rhs(hsb8, kblk),
                                start=(idx == 0),
                                stop=(idx == n - 1),
                                perf_mode=DR,
                            )
                            idx += 1
                    qsb = qk_pool.tile([P, S], BF16, name="qtsb", tag="qtsb")
                    nc.vector.tensor_copy(qsb, qps)
                    dst.append(qsb)

            v_aug = []
            for st in range(NST):
                vps = ps_mm.tile([P, D], F32, name="qps", tag="qps")
                for i in range(NIC):
                    nc.tensor.matmul(
                        vps,
                        lhsT=ht_all[:, st, i, :],
                        rhs=wv_sb[i],
                        start=(i == 0),
                        stop=(i == NIC - 1),
                    )
                vsb = v_pool.tile([P, H, 66], BF16, name="vsb", tag="vsb")
                nc.vector.memset(vsb[:, :, 64:66], 1.0)
                nc.vector.tensor_copy(
                    vsb[:, :, 0:64], vps.rearrange("p (h d) -> p h d", h=H)
                )
                v_aug.append(vsb)

            cxu = cxa_pool.tile([65, H, S], BF16, name="cxu", tag="cxu")
            for hp in range(H // 2):
                cps2 = [
                    ps_cx.tile([65, S], F32, name="cps", tag="cps") for _ in range(2)
                ]
                for ki in range(NST):
                    qoff = ki * P
                    nq = S - qoff
                    sps = ps_sc.tile([P, 2, S], F32, name="sps", tag="sps")
                    for e in range(2):
                        hr = e * DK
                        nc.tensor.matmul(
                            sps[:, e, 0:nq],
                            lhsT=kt_sb[hp][hr : hr + DK, ki * P : (ki + 1) * P],
                            rhs=qt_sb[hp][hr : hr + DK, qoff:S],
                            start=True,
                            stop=True,
                        )
                    pt = pt_pool.tile([P, 2, S], BF16, name="pt", tag="pt")
                    nc.scalar.activation(
                        pt[:, :, 0:nq], sps[:, :, 0:nq], AF.Exp, scale=EXP_SCALE
                    )
                    tsl = trim_sb[:, 1 if ki == 0 else 0, :]
                    tbc = bass.AP(
                        tensor=tsl.tensor,
                        offset=tsl.offset,
                        ap=[tsl.ap[0], [0, 2], [1, P]],
                    )
                    nc.vector.tensor_mul(pt[:, :, 0:P], pt[:, :, 0:P], tbc)
                    for e in range(2):
                        nc.tensor.matmul(
                            cps2[e][:, qoff:S],
                            lhsT=v_aug[ki][:, 2 * hp + e, 0:65],
                            rhs=pt[:, e, 0:nq],
                            start=(ki == 0),
                            stop=(ki == NST - 1),
                        )
                for e in range(2):
                    h = 2 * hp + e
                    # evacuate ctx + denominator row; bias adds 1e-10 only on
                    # row 64 (guards the zeroed q=0 reciprocal)
                    nc.scalar.activation(
                        cxu[:, h, :], cps2[e], AF.Identity, bias=tiny65, scale=1.0
                    )

            # shift each odd head's unnormalized ctx to partitions 64-127 now
            # (off the tail critical path) so the normalize muls are
            # partition-aligned with the pair-packed reciprocal tiles.
            shifts = []
            for hp in range(H // 2):
                csh = cx_pool.tile([P, S], BF16, name="csh", tag="csh", bufs=8)
                nc.sync.dma_start(out=csh[DK:P, :], in_=cxu[0:DK, 2 * hp + 1, :])
                shifts.append(csh)

            # denominator gather (row 64 across all heads) via gpsimd SWDGE;
            # the reciprocal + broadcasts run one pipeline stage later.
            dcat = dn_pool.tile([H, S], BF16, name="dcat", tag="dcat")
            nc.gpsimd.dma_start(out=dcat, in_=cxu[64:65, :, :])
            return (b, (cxu, shifts), dcat)

        def denom_chain(b, dcat):
            dc32 = dn_pool.tile([H, S], F32, name="dc32", tag="dc32")
            nc.vector.tensor_copy(dc32, dcat)
            rc32 = dn_pool.tile([H, S], F32, name="rc32", tag="rc32")
            nc.vector.reciprocal_approx_fast(out=rc32, in_=dc32)
            rcat = dn_pool.tile([H, S], BF16, name="rcat", tag="rcat")
            nc.vector.tensor_copy(rcat, rc32)
            rts = []
            for hp in range(H // 2):
                rt = r_pool.tile([P, S], BF16, name="rt", tag="rt")
                src = rcat[2 * hp : 2 * hp + 2, :]
                srcb = bass.AP(
                    tensor=src.tensor,
                    offset=src.offset,
                    ap=[src.ap[0], [0, DK], [1, S]],
                )
                nc.sync.dma_start(out=rt, in_=srcb)
                rts.append(rt)
            return rts

        def tail_norm(b, cxs, rts):
            # normalize into head-PAIR tiles [128, S]: even head on rows 0-63
            # from cxu; odd head on rows 64-127 from the pre-shifted staging.
            cxu, shifts = cxs
            pairs = []
            for hp in range(H // 2):
                csbp = cx_pool.tile([P, S], BF16, name="csbp", tag="csbp", bufs=8)
                nc.gpsimd.tensor_mul(
                    csbp[0:DK, :], cxu[0:DK, 2 * hp, :], rts[hp][0:DK, :]
                )
                nc.gpsimd.tensor_mul(
                    csbp[DK:P, :], shifts[hp][DK:P, :], rts[hp][DK:P, :]
                )
                pairs.append(csbp)
            return (b, pairs)

        def tail_mm(b, pairs):
            for st in range(NST):
                ops = ps_mm.tile([P, D], F32, name="qps", tag="qps")
                for hp in range(H // 2):
                    nc.tensor.matmul(
                        ops,
                        lhsT=pairs[hp][:, st * P : (st + 1) * P],
                        rhs=wo_sb[hp],
                        start=(hp == 0),
                        stop=(hp == H // 2 - 1),
                    )
                osb = o_pool.tile([P, D], F32, name="osb", tag="osb")
                nc.scalar.copy(osb, ops)
                nc.sync.dma_start(out=out[b, st * P : (st + 1) * P, :], in_=osb)

        # 5-deep software pipeline over batches: per iteration the engine
        # streams carry [conv(b) | outproj(b-4) | normalize(b-3) |
        # qk/v+attention(b-1) | denominator chain(b-2)].
        pend_mid = None
        pend_den = None
        pend_tail = None
        pend_norm = None
        x_cur = load_x(0)
        for b in range(BL):
            x_next = load_x(b + 1) if b + 1 < BL else None
            hts = front(b, x_cur)
            if pend_norm is not None:
                tail_mm(*pend_norm)
            new_norm = tail_norm(*pend_tail) if pend_tail is not None else None
            new_den = mid(*pend_mid) if pend_mid is not None else None
            if pend_den is not None:
                db, dcxu, ddcat = pend_den
                new_tail = (db, dcxu, denom_chain(db, ddcat))
            else:
                new_tail = None
            pend_mid = (b, hts)
            pend_den = new_den
            pend_tail = new_tail
            pend_norm = new_norm
            x_cur = x_next
        # drain
        if pend_norm is not None:
            tail_mm(*pend_norm)
        new_den = mid(*pend_mid)
        db, dcxu, ddcat = pend_den
        new_tail = (db, dcxu, denom_chain(db, ddcat))
        tail_mm(*tail_norm(*pend_tail))
        pend_den, pend_tail = new_den, new_tail
        db, dcxu, ddcat = pend_den
        new_tail = (db, dcxu, denom_chain(db, ddcat))
        tail_mm(*tail_norm(*pend_tail))
        tail_mm(*tail_norm(*new_tail))

    nc.compile()
    return nc


def _split_e4m3(a):
    e4 = ml_dtypes.float8_e4m3
    hi = np.asarray(a, np.float32).astype(e4)
    lo = (np.asarray(a, np.float32) - hi.astype(np.float32)).astype(e4)
    return hi, lo


def prep_inputs(inputs):
    """Host-side prep: shard over batch, fold scales into weights, build fp8
    hi/lo splits and the duplicated conv window layout."""
    x = np.asarray(inputs["x"], np.float32)
    conv_w = np.asarray(inputs["conv_w"], np.float32)
    conv_b = np.asarray(inputs["conv_b"], np.float32)
    sb = np.asarray(inputs["sqrt_beta"], np.float32).reshape(D)
    ln_w = np.asarray(inputs["ln_w"], np.float32)
    ln_b = np.asarray(inputs["ln_b"], np.float32)
    Wq = np.asarray(inputs["Wq"], np.float32)
    Wk = np.asarray(inputs["Wk"], np.float32)
    Wv = np.asarray(inputs["Wv"], np.float32)
    Wo = np.asarray(inputs["Wo"], np.float32)
    mask = np.asarray(inputs["mask"])

    for nm in ("bq", "bk", "bv", "bo"):
        assert not np.any(np.asarray(inputs[nm])), f"{nm} must be zero"
    assert not np.any(conv_b), "conv_b must be zero"
    assert not np.any(ln_b), "ln_b must be zero"
    assert np.array_equal(
        mask.reshape(S, S), np.tril(np.ones((S, S), mask.dtype))
    ), "mask must be causal"

    c1 = 1.0 - sb * sb
    c2 = 1.0 + sb * sb
    Wp = conv_w * c1[:, None, None]  # [o, i, k]
    Wp[np.arange(D), np.arange(D), 2] += c2
    Wp *= WC_SCALE
    wp_hi, wp_lo = _split_e4m3(Wp)
    e4 = ml_dtypes.float8_e4m3

    def arrange_wc(Wsplit):  # [o, i, k] e4m3 -> [128, 6, 2, D]
        wc = np.empty((P, NPAIR, 2, D), e4)
        for m in range(NPAIR):
            for j in range(2):
                i, k = BLOCKS[2 * m + j]
                # wc[p, m, j, o] = W[o, i*128+p, k]
                wc[:, m, j, :] = Wsplit[:, i * P : (i + 1) * P, k].T
        return wc

    wc_hi = arrange_wc(wp_hi)
    wc_lo = arrange_wc(wp_lo)

    def arrange_wqk(W):  # [o, i] -> hi/lo [128, kblk, oc, j, m]
        Wf = (W * ln_w[None, :]).T * WQK_SCALE  # [d_in, o]
        hi, lo = _split_e4m3(Wf)

        def arr(a):
            # a[d_in, o] -> [p, kblk, oc, jj, m]; d_in=(2*kblk+jj)*128+p, o=oc*128+m
            return np.ascontiguousarray(
                a.reshape(2, 2, P, NIC, P).transpose(2, 0, 3, 1, 4)
            )

        return arr(hi), arr(lo)

    wq_hi, wq_lo = arrange_wqk(Wq)
    wk_hi, wk_lo = arrange_wqk(Wk)

    bf = ml_dtypes.bfloat16

    def fold(W):  # [o, i] -> [ic, il, o] with ln_w folded on i
        Wf = W * ln_w[None, :]
        return np.ascontiguousarray(Wf.T).reshape(NIC, P, D)

    wv_h = fold(Wv)
    wo_h = np.ascontiguousarray(Wo.T).reshape(NIC, P, D)

    tri = np.triu(np.ones((P, P), np.float32))
    tri0 = tri.copy()
    tri0[:, 0] = 0.0
    trim = np.stack([tri, tri0], axis=1)  # [P, 2, P]

    consts = {
        "wc_hi": wc_hi,
        "wc_lo": wc_lo,
        "wq_hi": wq_hi,
        "wq_lo": wq_lo,
        "wk_hi": wk_hi,
        "wk_lo": wk_lo,
        "wv": wv_h.astype(bf),
        "wo": wo_h.astype(bf),
        "trim": trim.astype(bf),
    }

    in_maps = []
    for c in range(NCORES):
        xs = x[c * BL : (c + 1) * BL]  # [BL, S, D]
        xpad = np.zeros((BL, D, S + 2), np.float32)
        xpad[:, :, 2:] = xs.transpose(0, 2, 1)
        xp_hi, xp_lo = _split_e4m3(xpad)

        def windows(xp):  # [BL, D, S+2] -> [BL, 128, NST, 6, 2, 128]
            w = np.empty((BL, P, NST, NPAIR, 2, P), e4)
            for m in range(NPAIR):
                for j in range(2):
                    i, k = BLOCKS[2 * m + j]
                    blk = xp[:, i * P : (i + 1) * P, :]
                    for st in range(NST):
                        w[:, :, st, m, j, :] = blk[:, :, st * P + k : st * P + k + P]
            return w

        m = dict(consts)
        m["xdr_hi"] = windows(xp_hi)
        m["xdr_lo"] = windows(xp_lo)
        in_maps.append(m)
    return in_maps


_NC_CACHE = {}


def get_nc():
    if "nc" not in _NC_CACHE:
        _NC_CACHE["nc"] = build_nc()
    return _NC_CACHE["nc"]


def kernel(**inputs):
    nc = get_nc()
    in_maps = prep_inputs(inputs)
    res = run_bass_kernel_spmd(nc, in_maps, list(range(NCORES)))
    outs = [np.asarray(r["out"], np.float32) for r in res.results]
    return np.concatenate(outs, axis=0)


if __name__ == "__main__":
    nc = build_nc()
    print("built ok")
